# revision 1
# baseline (speedup 1.0000x reference)
"""Trainium2 Bass kernel for the CMB power-spectrum emulator problem.

Math: a 4-layer MLP maps phi (512,2) -> diag (128 knots, 512 ch); a natural
cubic spline through the 128 knots is evaluated on a constant 256x256
isotropic-frequency grid, then exp(.)*NORM.

Two structural collapses, both input-independent:
 1. The spline is linear in the knot values, so the whole spline stage is
    one constant matrix E:  out = exp(E @ diag + ln NORM).
 2. The grid value wn_iso[i,j] depends only on (a,b) = sorted(|wn_i|,|wn_j|),
    an exact 8-fold dihedral symmetry: only 8385 of the 65536 grid points
    are distinct, and equal points produce bitwise-equal outputs. The device
    computes the 8385 unique points; the host replicates them with a
    constant gather.

Device work per core (unique-point sharding, 1056 points/core, 512 ch):
  MLP as two interleaved 256-wide chains (f32r matmuls on TensorE,
    relu+bias and the final bias-add on the otherwise-idle VectorE,
    keeping the ScalarE FIFO clear for the exp stream) -> diag (128, 512)
  per 128-channel group g: psum = diag_g.T @ ET_u  (TensorE, f32r)
                           stage = exp(psum+lnN)   (ScalarE LUT, ~2 ULP)
                           store (128, 1056) fp32  (SP HWDGE ring)
"""

import os

import numpy as np

B = 512
N_CORES = 8
N_UNIQ = 129 * 130 // 2       # 8385 distinct grid values
P_CORE = 1056                 # per-core unique points (8 x 1056 = 8448 padded)
P_PAD = N_CORES * P_CORE
NORM = 1.0 / 12661.0

MIN_PHI = np.array([50.0, 0.0075], np.float32)
DPHI = np.array([40.0, 0.0492], np.float32)
MU = np.array([70.0, 0.032], np.float32)
SIG = np.array([20.0, 0.025], np.float32)

# matmul dtype: "f32" (4 cyc/row, exact), "f32r" (1 cyc/row, ~19-bit mantissa)
MODE = os.environ.get("BASS_KERNEL_MODE", "f32r")

# packed parameters: ph (2 partitions: phiT|W1), pm (128p: W2|W3|W4)
PH_COLS = 612
PM_COLS = 328
PB_COLS = 5  # fp32 part: b1, b2, b3, b4, ln(NORM)

_CACHE = {}


def _spline_eval_matrix(wn_vals):
    """E (len(wn_vals), 128) fp32: natural-cubic-spline evaluation at wn_vals,
    linear in the 128 knot values (knots t_k = sqrt(2)*k in fp32)."""
    wn = (256.0 * np.fft.fftfreq(256, d=1.0)).reshape(256, 1)
    wn_iso = np.sqrt(wn**2 + wn.reshape(1, 256) ** 2)
    t32 = np.fft.fftshift(wn_iso).diagonal()[128:].astype(np.float32)  # (128,)

    n = 128
    t = t32.astype(np.float64)
    h = np.diff(t)
    A = np.diag(2.0 * (h[:-1] + h[1:])) + np.diag(h[1:-1], 1) + np.diag(h[1:-1], -1)
    D1 = np.zeros((n - 1, n))
    for i in range(n - 1):
        D1[i, i] = -1.0 / h[i]
        D1[i, i + 1] = 1.0 / h[i]
    D2 = 6.0 * (D1[1:] - D1[:-1])
    L = np.zeros((n, n))
    L[1:-1] = np.linalg.solve(A, D2)

    Sa = np.eye(n)[: n - 1]
    Sb = D1 - (h[:, None] / 6.0) * (2.0 * L[:-1] + L[1:])
    Sc = L[:-1] / 2.0
    Sd = (L[1:] - L[:-1]) / (6.0 * h[:, None])

    w32 = wn_vals.astype(np.float32)
    idx = np.clip(np.searchsorted(t32, w32, side="right") - 1, 0, n - 2)
    f = (w32 - t32[idx]).astype(np.float64)[:, None]
    E = Sa[idx] + f * (Sb[idx] + f * (Sc[idx] + f * Sd[idx]))
    return E.astype(np.float32)


def _build_constants():
    """ET_u (128, P_PAD) fp32 for the unique points, and IDX (65536,) int32
    mapping each full-grid point to its unique-point column."""
    k = np.arange(256)
    absw = np.minimum(k, 256 - k)  # |wn_i|, with |wn_0| = 0, |wn_128| = 128
    ai = np.minimum(absw[:, None], absw[None, :])
    bi = np.maximum(absw[:, None], absw[None, :])
    uid = (bi * (bi + 1)) // 2 + ai  # (256,256) in [0, N_UNIQ)

    bs = np.concatenate([np.full(b + 1, b) for b in range(129)])  # uid -> b
    as_ = np.concatenate([np.arange(b + 1) for b in range(129)])  # uid -> a
    wn_vals = np.sqrt((as_.astype(np.float64)) ** 2 + (bs.astype(np.float64)) ** 2)

    E = _spline_eval_matrix(wn_vals)  # (8385, 128)
    ET = np.zeros((128, P_PAD), np.float32)
    ET[:, :N_UNIQ] = E.T
    return np.ascontiguousarray(ET), uid.ravel().astype(np.int32)


def _build_program(mode):
    import concourse.bass as bass
    import concourse.bacc as bacc
    import concourse.mybir as mybir
    from concourse import tile

    f32 = mybir.dt.float32
    mm_dt = {"f32r": mybir.dt.float32r, "f32": f32, "mix": mybir.dt.float32r}[mode]
    main_dt = mybir.dt.bfloat16 if mode == "mix" else mm_dt
    nc = bacc.Bacc("TRN2", target_bir_lowering=False, debug=False)

    ph_d = nc.dram_tensor("ph", [2, PH_COLS], mm_dt, kind="ExternalInput")
    pm_d = nc.dram_tensor("pm", [128, PM_COLS], mm_dt, kind="ExternalInput")
    pb_d = nc.dram_tensor("pb", [128, PB_COLS], f32, kind="ExternalInput")
    et_d = nc.dram_tensor("et", [128, P_CORE], main_dt, kind="ExternalInput")
    out_d = nc.dram_tensor("out", [B, P_CORE], f32, kind="ExternalOutput")

    Relu = mybir.ActivationFunctionType.Relu
    Ident = mybir.ActivationFunctionType.Identity
    Exp = mybir.ActivationFunctionType.Exp

    N_GRP = 4
    SUB = 512  # matmul free chunk (PSUM bank)

    with tile.TileContext(nc) as tc:
        with (
            tc.tile_pool(name="const", bufs=1) as cpool,
            tc.tile_pool(name="mlp", bufs=2) as mpool,
            tc.tile_pool(name="stage", bufs=4) as spool,
            tc.tile_pool(name="psum", bufs=2, space=bass.MemorySpace.PSUM) as ppool,
            tc.tile_pool(name="mpsum", bufs=2, space=bass.MemorySpace.PSUM) as mps,
        ):
            # ---- loads on the idle SP ring: params first, then ET ----
            ph_t = cpool.tile([2, PH_COLS], mm_dt, tag="ph")
            nc.sync.dma_start(ph_t[:], ph_d[:])
            pm_t = cpool.tile([128, PM_COLS], mm_dt, tag="pm")
            nc.sync.dma_start(pm_t[:], pm_d[:])
            pb_t = cpool.tile([128, PB_COLS], f32, tag="pb")
            nc.sync.dma_start(pb_t[:], pb_d[:])
            et_t = cpool.tile([128, P_CORE], main_dt, tag="et")
            nc.sync.dma_start(et_t[:], et_d[:])

            pht = ph_t[0:2, 0:512]
            w1 = ph_t[0:2, 512:612]
            w2 = pm_t[0:100, 0:100]
            w3 = pm_t[0:100, 100:200]
            w4 = pm_t[0:100, 200:328]
            b1 = pb_t[0:100, 0:1]
            b2 = pb_t[0:100, 1:2]
            b3 = pb_t[0:100, 2:3]
            b4 = pb_t[0:128, 3:4]
            lnb = pb_t[0:128, 4:5]


            # ---- MLP, two interleaved 256-wide chains (hides sem latency) ----
            HB = B // 2
            diag = mpool.tile([128, B], main_dt, tag="diag")
            hs = {}
            for lyr, (wt, bt, act, win, wout) in enumerate(
                [
                    (w1, b1, Relu, 2, 100),
                    (w2, b2, Relu, 100, 100),
                    (w3, b3, Relu, 100, 100),
                    (w4, b4, Ident, 100, 128),
                ]
            ):
                for c in range(2):
                    cs = slice(c * HB, (c + 1) * HB)
                    src = pht[:, cs] if lyr == 0 else hs[c][:]
                    ps = mps.tile([128, SUB], f32, tag="mps")
                    nc.tensor.matmul(ps[0:wout, 0:HB], wt, src)
                    if lyr < 3:
                        h = mpool.tile([100, HB], mm_dt, tag=f"h{lyr}{c}")
                        nc.vector.tensor_scalar(
                            h[:], ps[0:wout, 0:HB], bt, 0.0,
                            mybir.AluOpType.add, mybir.AluOpType.max,
                        )
                        hs[c] = h
                    else:
                        nc.vector.tensor_scalar(
                            diag[:, cs], ps[0:wout, 0:HB], bt, None,
                            mybir.AluOpType.add,
                        )

            # ---- main: out[g] = exp(diag_g.T @ ET_u + lnN), one store per g;
            # last group split in two so its store starts sooner ----
            for g in range(N_GRP):
                ps = ppool.tile([128, P_CORE], f32, tag="ps")
                for off in range(0, P_CORE, SUB):
                    w = min(SUB, P_CORE - off)
                    nc.tensor.matmul(
                        ps[:, off : off + w],
                        diag[:, g * 128 : (g + 1) * 128],
                        et_t[:, off : off + w],
                    )
                stage = spool.tile([128, P_CORE], f32, tag="stage")
                orow = out_d[g * 128 : (g + 1) * 128, :]
                if g < N_GRP - 1:
                    nc.scalar.activation(stage[:], ps[:], Exp, bias=lnb)
                    nc.sync.dma_start(orow, stage[:])
                else:
                    hp = 800  # late split: small final exp+store
                    nc.scalar.activation(stage[:, :hp], ps[:, :hp], Exp, bias=lnb)
                    nc.sync.dma_start(orow[:, :hp], stage[:, :hp])
                    nc.scalar.activation(stage[:, hp:], ps[:, hp:], Exp, bias=lnb)
                    nc.sync.dma_start(orow[:, hp:], stage[:, hp:])

    nc.compile()
    return nc


def _get_cached():
    key = ("nc", MODE)
    if key not in _CACHE:
        _CACHE[key] = _build_program(MODE)
    if "consts" not in _CACHE:
        _CACHE["consts"] = _build_constants()
    return (_CACHE[key],) + _CACHE["consts"]


def _make_in_maps(phi, W1, b1, W2, b2, W3, b3, W4, b4, ET):
    # fold the input normalization into the first layer
    scale = (DPHI / SIG).astype(np.float32)
    shift = ((MIN_PHI - MU) / SIG).astype(np.float32)
    W1f = (np.asarray(W1, np.float32) * scale[:, None]).astype(np.float32)
    b1f = (np.asarray(b1, np.float32) + shift @ np.asarray(W1, np.float32)).astype(
        np.float32
    )

    ph = np.zeros((2, PH_COLS), np.float32)
    ph[:, 0:512] = np.asarray(phi, np.float32).T
    ph[:, 512:612] = W1f
    pm = np.zeros((128, PM_COLS), np.float32)
    pm[0:100, 0:100] = np.asarray(W2, np.float32)
    pm[0:100, 100:200] = np.asarray(W3, np.float32)
    pm[0:100, 200:328] = np.asarray(W4, np.float32)
    pb = np.zeros((128, PB_COLS), np.float32)
    pb[0:100, 0] = np.asarray(b1f, np.float32)
    pb[0:100, 1] = np.asarray(b2, np.float32)
    pb[0:100, 2] = np.asarray(b3, np.float32)
    pb[0:128, 3] = np.asarray(b4, np.float32)
    pb[:, 4] = np.log(np.float64(NORM))

    common = {"ph": ph, "pm": pm, "pb": pb}
    in_maps = []
    for c in range(N_CORES):
        m = dict(common)
        shard = np.ascontiguousarray(ET[:, c * P_CORE : (c + 1) * P_CORE])
        if MODE == "mix":
            import ml_dtypes

            shard = shard.astype(ml_dtypes.bfloat16)
        m["et"] = shard
        in_maps.append(m)
    return in_maps


def kernel(phi, W1, b1, W2, b2, W3, b3, W4, b4):
    from concourse.bass_utils import run_bass_kernel_spmd

    nc, ET, IDX = _get_cached()
    in_maps = _make_in_maps(phi, W1, b1, W2, b2, W3, b3, W4, b4, ET)
    res = run_bass_kernel_spmd(nc, in_maps, core_ids=list(range(N_CORES)))
    uniq = np.concatenate([r["out"] for r in res.results], axis=1)  # (512, 8448)
    full = np.take(uniq, IDX, axis=1)  # (512, 65536) constant-gather replication
    return np.ascontiguousarray(full.reshape(B, 256, 256))



# revision 12
# speedup vs baseline: 1.0272x; 1.0272x over previous
"""Trainium2 Bass kernel for the CMB power-spectrum emulator problem.

Math: a 4-layer MLP maps phi (512,2) -> diag (128 knots, 512 ch); a natural
cubic spline through the 128 knots is evaluated on a constant 256x256
isotropic-frequency grid, then exp(.)*NORM.

Structural collapses (all input-independent or host-cheap):
 1. The spline is linear in the knot values: the whole spline stage is a
    constant matrix E (grid_points, 128) applied to the knot values.
 2. The grid has an exact 8-fold dihedral symmetry: only 8385 of the 65536
    grid points are distinct; the device computes the unique points and the
    host replicates them with a constant gather.
 3. The last (linear) MLP layer commutes with E: G = E @ W4.T (P,100) and
    c = E @ b4 + ln NORM are folded on the host, so the device runs only the
    3 relu layers and one (102-row) matmul per point block. The per-point
    bias c rides inside the matmul as two bf16 hi/lo rows multiplied by
    constant-1 rows of the stationary operand (bf16 alone would lose the
    ~9.45 magnitude of ln NORM; the hi/lo split restores ~16-bit precision).

Device work per core (point sharding, 1056 points, 512 channels, all bf16):
  junk matmuls warm the PE HAM clock-gate during the input-DMA window;
  a dummy exp preloads the ACT table set at t=0.
  MLP as two interleaved 256-wide chains -> h3 (100, 512) bf16 (+ ones rows)
  per 128-channel group g: psum = h3_g.T @ GT   (TensorE, bf16, K=102)
                           stage = exp(psum)    (ScalarE LUT)
                           store (128, 1056) bf16 (DMA, issue spread over
                           sync/gpsimd/vector queues)
Host: upcast bf16->f32, constant gather to (512, 256, 256).
"""

import os

import ml_dtypes
import numpy as np

B = 512
N_CORES = 8
N_UNIQ = 129 * 130 // 2       # 8385 distinct grid values
P_CORE = 1056                 # per-core unique points (8 x 1056 = 8448 padded)
P_PAD = N_CORES * P_CORE
NORM = 1.0 / 12661.0
HB = B // 2                   # per-chain batch width

MIN_PHI = np.array([50.0, 0.0075], np.float64)
DPHI = np.array([40.0, 0.0492], np.float64)
MU = np.array([70.0, 0.032], np.float64)
SIG = np.array([20.0, 0.025], np.float64)

PA_COLS = 612   # phiT (512) | W1f (100); row 2 = ones | b1f (bias fold)
PA_ROWS = 3
PW_COLS = 200   # W2 (100) | W3 (100); row 100 = b2 | b3 (bias fold)
PW_ROWS = 101
GT_ROWS = 102   # G.T (100) ; c_hi ; c_lo

JUNK_PRE = int(os.environ.get("BK_JUNK_PRE", "12"))   # warmup MMs before MLP
JUNK_MLP = int(os.environ.get("BK_JUNK_MLP", "1"))    # filler MMs per MLP MM
SPLIT = int(os.environ.get("BK_SPLIT", "800"))        # last-group exp split

_CACHE = {}


def _bf16(x):
    return np.asarray(x, np.float64).astype(ml_dtypes.bfloat16)


def _spline_eval_matrix_f64(wn_vals):
    """E (len(wn_vals), 128) f64: natural-cubic-spline evaluation at wn_vals,
    linear in the 128 knot values (knots t_k = sqrt(2)*k in fp32)."""
    wn = (256.0 * np.fft.fftfreq(256, d=1.0)).reshape(256, 1)
    wn_iso = np.sqrt(wn**2 + wn.reshape(1, 256) ** 2)
    t32 = np.fft.fftshift(wn_iso).diagonal()[128:].astype(np.float32)  # (128,)

    n = 128
    t = t32.astype(np.float64)
    h = np.diff(t)
    A = np.diag(2.0 * (h[:-1] + h[1:])) + np.diag(h[1:-1], 1) + np.diag(h[1:-1], -1)
    D1 = np.zeros((n - 1, n))
    for i in range(n - 1):
        D1[i, i] = -1.0 / h[i]
        D1[i, i + 1] = 1.0 / h[i]
    D2 = 6.0 * (D1[1:] - D1[:-1])
    L = np.zeros((n, n))
    L[1:-1] = np.linalg.solve(A, D2)

    Sa = np.eye(n)[: n - 1]
    Sb = D1 - (h[:, None] / 6.0) * (2.0 * L[:-1] + L[1:])
    Sc = L[:-1] / 2.0
    Sd = (L[1:] - L[:-1]) / (6.0 * h[:, None])

    w32 = wn_vals.astype(np.float32)
    idx = np.clip(np.searchsorted(t32, w32, side="right") - 1, 0, n - 2)
    f = (w32 - t32[idx]).astype(np.float64)[:, None]
    return Sa[idx] + f * (Sb[idx] + f * (Sc[idx] + f * Sd[idx]))  # f64


def _build_constants():
    """E64 (P_PAD, 128) f64 spline-eval matrix at the unique points (zero
    rows for padding), and IDX (65536,) int32 full-grid -> unique column."""
    k = np.arange(256)
    absw = np.minimum(k, 256 - k)
    ai = np.minimum(absw[:, None], absw[None, :])
    bi = np.maximum(absw[:, None], absw[None, :])
    uid = (bi * (bi + 1)) // 2 + ai  # (256,256) in [0, N_UNIQ)

    bs = np.concatenate([np.full(b + 1, b) for b in range(129)])
    as_ = np.concatenate([np.arange(b + 1) for b in range(129)])
    wn_vals = np.sqrt(as_.astype(np.float64) ** 2 + bs.astype(np.float64) ** 2)

    E = np.zeros((P_PAD, 128), np.float64)
    E[:N_UNIQ] = _spline_eval_matrix_f64(wn_vals)
    return E, uid.ravel().astype(np.int32)


def _build_program():
    import concourse.bass as bass
    import concourse.bacc as bacc
    import concourse.mybir as mybir
    from concourse import tile

    f32 = mybir.dt.float32
    bf16 = mybir.dt.bfloat16
    nc = bacc.Bacc("TRN2", target_bir_lowering=False, debug=False)

    pa_d = nc.dram_tensor("pa", [PA_ROWS, PA_COLS], bf16, kind="ExternalInput")
    pw_d = nc.dram_tensor("pw", [PW_ROWS, PW_COLS], bf16, kind="ExternalInput")
    gt_d = nc.dram_tensor("gt", [GT_ROWS, P_CORE], bf16, kind="ExternalInput")
    out_d = nc.dram_tensor("out", [B, P_CORE], bf16, kind="ExternalOutput")

    Exp = mybir.ActivationFunctionType.Exp
    Max = mybir.AluOpType.max

    N_GRP = 4
    SUB = 512  # matmul free chunk (PSUM bank)

    with tile.TileContext(nc) as tc:
        with (
            tc.tile_pool(name="const", bufs=1) as cpool,
            tc.tile_pool(name="stage", bufs=4) as spool,
            tc.tile_pool(name="psum", bufs=2, space=bass.MemorySpace.PSUM) as ppool,
            tc.tile_pool(name="mpsum", bufs=2, space=bass.MemorySpace.PSUM) as mps,
        ):
            # ---- tiles ----
            pa_t = cpool.tile([PA_ROWS, PA_COLS], bf16, tag="pa")
            pw_t = cpool.tile([PW_ROWS, PW_COLS], bf16, tag="pw")
            gt_t = cpool.tile([GT_ROWS, P_CORE], bf16, tag="gt")
            jt = cpool.tile([128, 256], bf16, tag="jt")
            e1 = cpool.tile([1, 8], f32, tag="e1")
            e1o = cpool.tile([1, 8], f32, tag="e1o")
            h3a = cpool.tile([GT_ROWS, HB], bf16, tag="h3a")
            h3b = cpool.tile([GT_ROWS, HB], bf16, tag="h3b")
            hts = {
                (l, c): cpool.tile(
                    [PW_ROWS, HB], bf16, tag=f"h{l}{c}", name=f"h{l}{c}"
                )
                for l in range(2)
                for c in range(2)
            }
            # junk-MM target: first slot of the main psum ring (recycled by
            # group 1 much later; PE program order makes the reuse safe)
            jps = ppool.tile([128, P_CORE], f32, tag="ps", name="jps")

            # ---- input loads, issue spread across idle engine queues; the
            # gt issue goes on scalar BEFORE the exp-table preload ----
            nc.sync.dma_start(pa_t[:], pa_d[:])
            nc.gpsimd.dma_start(pw_t[:], pw_d[:])
            nc.scalar.dma_start(gt_t[:], gt_d[:])

            # ---- preload the exp table set on ScalarE; init consts ----
            nc.vector.memset(e1[:], 0.0)
            nc.scalar.activation(e1o[:], e1[:], Exp)
            # ones rows for the bias/c folds. Engine APs must start at a
            # partition in {0,32,64,96}, so memset from 96; the relu writes
            # to [0:100] later overwrite rows 96..99 with the real values.
            nc.gpsimd.memset(jt[:], 0.0)
            nc.vector.memset(h3a[96:102, :], 1.0)
            nc.vector.memset(h3b[96:102, :], 1.0)
            for t in hts.values():
                nc.vector.memset(t[96:101, :], 1.0)

            # ---- warm the PE (HAM clock gate) while the inputs land ----
            for _ in range(JUNK_PRE):
                nc.tensor.matmul(jps[:, 0:128], jt[:, 0:128], jt[:, 128:256])

            w1 = pa_t[0:3, 512:612]     # row 2 = b1f
            w2 = pw_t[:, 0:100]          # row 100 = b2
            w3 = pw_t[:, 100:200]        # row 100 = b3

            # ---- MLP, two interleaved 256-wide chains; biases ride in the
            # matmuls via ones rows; junk MMs keep the PE busy through the
            # chain's engine-handoff gaps ----
            for lyr, wt in enumerate([w1, w2, w3]):
                for c in range(2):
                    cs = slice(c * HB, (c + 1) * HB)
                    src = pa_t[0:3, cs] if lyr == 0 else hts[(lyr - 1, c)][:]
                    ps = mps.tile([128, 256], f32, tag="mps")
                    nc.tensor.matmul(ps[0:100, 0:HB], wt, src)
                    for _ in range(JUNK_MLP):
                        nc.tensor.matmul(
                            jps[:, 0:128], jt[:, 0:128], jt[:, 128:256]
                        )
                    dst = hts[(lyr, c)][0:100, :] if lyr < 2 else (
                        (h3a if c == 0 else h3b)[0:100, :]
                    )
                    nc.vector.tensor_scalar(
                        dst, ps[0:100, 0:HB], 0.0, None, Max
                    )

            # ---- main: out[g] = exp(h3aug_g.T @ GTaug), one store per g;
            # last group split so its store starts sooner. Store issues are
            # rotated across engine DMA queues. ----
            store_eng = [nc.sync, nc.gpsimd, nc.sync, nc.gpsimd, nc.sync]
            si = 0
            for g in range(N_GRP):
                h3 = h3a if g < 2 else h3b
                lhsT = h3[:, (g % 2) * 128 : (g % 2 + 1) * 128]
                ps = ppool.tile([128, P_CORE], f32, tag="ps")
                for off in range(0, P_CORE, SUB):
                    w = min(SUB, P_CORE - off)
                    nc.tensor.matmul(
                        ps[:, off : off + w], lhsT, gt_t[:, off : off + w]
                    )
                stage = spool.tile([128, P_CORE], bf16, tag="stage")
                orow = out_d[g * 128 : (g + 1) * 128, :]
                if g < N_GRP - 1:
                    nc.scalar.activation(stage[:], ps[:], Exp)
                    store_eng[si].dma_start(orow, stage[:])
                    si += 1
                else:
                    hp = SPLIT
                    nc.scalar.activation(stage[:, :hp], ps[:, :hp], Exp)
                    store_eng[si].dma_start(orow[:, :hp], stage[:, :hp])
                    si += 1
                    nc.scalar.activation(stage[:, hp:], ps[:, hp:], Exp)
                    store_eng[si].dma_start(orow[:, hp:], stage[:, hp:])

    nc.compile()
    return nc


def _get_cached():
    if "nc" not in _CACHE:
        _CACHE["nc"] = _build_program()
    if "consts" not in _CACHE:
        _CACHE["consts"] = _build_constants()
    return (_CACHE["nc"],) + _CACHE["consts"]


def _make_in_maps(phi, W1, b1, W2, b2, W3, b3, W4, b4, E64):
    # fold the input normalization into the first layer (f64 host math)
    scale = DPHI / SIG
    shift = (MIN_PHI - MU) / SIG
    W1_64 = np.asarray(W1, np.float64)
    W1f = W1_64 * scale[:, None]
    b1f = np.asarray(b1, np.float64) + shift @ W1_64

    pa = np.zeros((PA_ROWS, PA_COLS), ml_dtypes.bfloat16)
    pa[0:2, 0:512] = _bf16(np.asarray(phi, np.float64).T)
    pa[2, 0:512] = _bf16(1.0)                        # ones row (bias fold)
    pa[0:2, 512:612] = _bf16(W1f)
    pa[2, 512:612] = _bf16(b1f)
    pw = np.zeros((PW_ROWS, PW_COLS), ml_dtypes.bfloat16)
    pw[0:100, 0:100] = _bf16(W2)
    pw[0:100, 100:200] = _bf16(W3)
    pw[100, 0:100] = _bf16(b2)
    pw[100, 100:200] = _bf16(b3)

    # fold layer 4 into the spline-eval matrix: G (P,100), c (P,)
    G = E64 @ np.asarray(W4, np.float64).T          # (P_PAD, 100)
    c = E64 @ np.asarray(b4, np.float64) + np.log(np.float64(NORM))
    c_hi = _bf16(c)
    c_lo = _bf16(c - c_hi.astype(np.float64))
    GTb = _bf16(G.T)                                 # (100, P_PAD)

    common = {"pa": pa, "pw": pw}
    in_maps = []
    for cix in range(N_CORES):
        sl = slice(cix * P_CORE, (cix + 1) * P_CORE)
        gt = np.zeros((GT_ROWS, P_CORE), ml_dtypes.bfloat16)
        gt[0:100] = GTb[:, sl]
        gt[100] = c_hi[sl]
        gt[101] = c_lo[sl]
        m = dict(common)
        m["gt"] = gt
        in_maps.append(m)
    return in_maps


def kernel(phi, W1, b1, W2, b2, W3, b3, W4, b4):
    from concourse.bass_utils import run_bass_kernel_spmd

    nc, E64, IDX = _get_cached()
    in_maps = _make_in_maps(phi, W1, b1, W2, b2, W3, b3, W4, b4, E64)
    res = run_bass_kernel_spmd(nc, in_maps, core_ids=list(range(N_CORES)))
    uniq = np.concatenate(
        [r["out"].astype(np.float32) for r in res.results], axis=1
    )  # (512, 8448) f32
    full = np.take(uniq, IDX, axis=1)  # (512, 65536) constant-gather replication
    return np.ascontiguousarray(full.reshape(B, 256, 256))
